# revision 56
# baseline (speedup 1.0000x reference)
"""Causal self-attention block (QKV -> causal attention -> 2 projections)
distributed over 8 NeuronCores via Bass/Tile.

Sharding: tensor-parallel over heads (2 heads/core, both batches on every
core). The whole on-device computation runs in transposed activation space
(channel/head-dim on partitions, tokens on the free axis) so no on-device
transposes of activations are ever needed.

Token order within each 512-token window is REVERSED (host-side, in x^T).
In reversed coordinates the causal mask becomes "q' <= k'", so every
diagonal-window narrowing is PREFIX-aligned:
  - scores for diagonal k-chunk kcl only need q' columns [0, 128*(kcl+1)),
    written at the PSUM bank start (PE PSUM writes must start banks);
  - the PV accumulation is narrowed the same way (diagonal chunks processed
    in DESCENDING kcl order so the first, full-width matmul initializes the
    accumulator region);
  - only one 128x128 upper-triangular block per diagonal chunk needs
    masking (done with one strided tensor_mul per chunk pair on DVE).

phase 1: Q^T, K^T = w^T @ x^T; V in [token, dim] orientation with an
         appended ones column (softmax row-sums for free in phase 2).
phase 2: S^T = K Q^T per 128k x 512q chunk, P = exp(S^T) on ScalarE,
         att^T accumulated on PE, normalized by the row-sum reciprocal
         (DVE recip -> Pool partition_broadcast -> DVE stt).
A2A:     three 8-rank AllToAlls (fp16) move att^T head-blocks so each
         core ends up owning 512 token rows of all 16 heads: group 0 =
         batch 0 (issued as soon as batch 0's attention is done, hidden
         under batch-1 compute), groups 1/2 = batch-1 window pairs so
         the only tail-exposed transfers are small.
phase 3: out^T = W3^T att^T with W3 = (wp1 . wp2) folded on the host;
         ec-outer loops per A2A group; group 0's matmuls are anchored
         into attention-7's exp-bound tail.

Work is emitted through a fine-grained unit interleave (attention window w
merged with phase-1 window w+2) so PE never waits on ScalarE exp; a few
warm-up matmuls hold the PE p-state up while the first DMAs land. Nothing
that waits on a collective may be scheduled inside the attention stream
(engine SEQs evaluate waits in-order), so phase-3 work is pinned late via
explicit no-sync anchor deps on attention-7 units.
"""

import os

import numpy as np

import concourse.bass as bass
import concourse.tile as tile
from concourse import bacc, mybir
from concourse.bass_utils import run_bass_kernel_spmd

B, T, C, H, D = 2, 2048, 1024, 16, 64
NCORES = 8
HPC = H // NCORES          # heads per core = 2
TT = B * T                 # 4096 flat (b, t) rows
W = 512                    # token window / q chunk
NW = TT // W               # 8 windows
WPB = T // W               # 4 windows per batch
QS = TT // NCORES          # 512 rows per core after A2A
HQS = QS // 2              # 256-token half window (batch-0 A2A payload)
QTR = QS // 4              # 128-token quarter (batch-1 A2A payloads)
NG = 3                     # collective groups: {b0}, {w4,w5}, {w6,w7}
GCW = (HQS, QTR, QTR)      # slot cols per collective group
GCOL = (0, HQS, HQS + QTR)  # out_t col offset per group
NCI = C // 128             # 8 channel chunks
KC = 128                   # k chunk
F32 = mybir.dt.float32
F32R = mybir.dt.float32r
BF16 = mybir.dt.float16
EXP = mybir.ActivationFunctionType.Exp
MULT = mybir.AluOpType.mult

_NC_CACHE = {}

KNOCC = int(os.environ.get("KNOCC", "0"))
KREP = int(os.environ.get("KREP", "1"))
KDBG = os.environ.get("KDBG", "")


def build_nc(krep=None, knocc=None):
    krep = KREP if krep is None else krep
    knocc = KNOCC if knocc is None else knocc
    key = (krep, knocc)
    if key in _NC_CACHE:
        return _NC_CACHE[key]
    nc = bacc.Bacc("TRN2", target_bir_lowering=False, debug=False,
                   num_devices=NCORES)
    xt_in = nc.dram_tensor("xt", [128, NW, NCI, W], BF16, kind="ExternalInput")
    wq_in = nc.dram_tensor("wq", [128, NCI, 128], BF16, kind="ExternalInput")
    wk_in = nc.dram_tensor("wk", [128, NCI, 128], BF16, kind="ExternalInput")
    wv_in = nc.dram_tensor("wv", [128, NCI, 128], BF16, kind="ExternalInput")
    w3_in = nc.dram_tensor("w3", [128, NCORES, C], BF16, kind="ExternalInput")
    tri_in = nc.dram_tensor("tri", [128, KC], BF16, kind="ExternalInput")
    vones_in = nc.dram_tensor("vones", [128, 4, 2], BF16, kind="ExternalInput")
    out_t = nc.dram_tensor("out_t", [C, QS], F32, kind="ExternalOutput")

    with tile.TileContext(nc) as tc:
      for rep in range(krep):
        with (
            tc.tile_pool(name=f"dramp{rep}", bufs=1, space="DRAM") as dramp,
            tc.tile_pool(name=f"dramq{rep}", bufs=1, space="DRAM") as dramq,
            tc.tile_pool(name=f"misc{rep}", bufs=1) as misc,
            tc.tile_pool(name=f"rcpool{rep}", bufs=4) as rcpool,
            tc.tile_pool(name=f"qtp{rep}", bufs=NW) as qtp,
            tc.tile_pool(name=f"ktp{rep}", bufs=NW) as ktp,
            tc.tile_pool(name=f"vp{rep}", bufs=NW) as vp,
            tc.tile_pool(name=f"attp{rep}", bufs=NW) as attp,
            tc.tile_pool(name=f"pbp{rep}", bufs=4) as pbp,
            tc.tile_pool(name=f"wqkv{rep}", bufs=1) as wqkv,
            tc.tile_pool(name=f"xtp{rep}", bufs=4) as xtp,
            tc.tile_pool(name=f"wpp{rep}", bufs=1) as wpp,
            tc.tile_pool(name=f"finp{rep}", bufs=NCORES) as finp,
            tc.tile_pool(name=f"otp{rep}", bufs=4) as otp,
            ):
            # A2A bounce buffers. Group 0 = batch 0 (slot r = (window
            # r//2, half r%2), 256 cols); groups 1/2 = batch-1 window
            # pairs {w4,w5}/{w6,w7} (slot r = (window r//4, quarter r%4),
            # 128 cols) so the last, tail-exposed collectives are small.
            pools = (dramp, dramq, dramq)
            cc_in = [pools[g].tile([NCORES, 128, GCW[g]], BF16,
                                   name=f"cin{g}", tag=f"cin{g}")
                     for g in range(NG)]
            cc_out = [pools[g].tile([NCORES, 128, GCW[g]], BF16,
                                    name=f"cout{g}", tag=f"cout{g}")
                      for g in range(NG)]

            tri_t = misc.tile([128, KC], BF16, name="tri_t")
            vones_t = misc.tile([128, 4, 2], BF16, name="vones_t")
            warm_t = misc.tile([128, 128], BF16, name="warm_t")
            wqt = wqkv.tile([128, NCI, 128], BF16, name="wqt")
            wkt = wqkv.tile([128, NCI, 128], BF16, name="wkt")
            wvt = wqkv.tile([128, NCI, 128], BF16, name="wvt")
            w3t = wpp.tile([128, NCORES, C], BF16, name="w3t")

            qt_tiles, kt_tiles, v_tiles, att_tiles = {}, {}, {}, {}

            dbgkeep = []
            if KDBG in ("prep", "rs"):
                dbgkeep = [misc.tile([128, W], F32, name=f"dk{i}")
                           for i in range(NW)]

            # phase 1 + phase 2 share one PSUM budget (8 banks):
            # qkv/ph3 ring 2 x 1 + scores 2 x 2 + psas 2 x 1 = 8
            with (
                tc.tile_pool(name=f"pqk{rep}", bufs=2, space="PSUM") as pqk,
                tc.tile_pool(name=f"pp{rep}", bufs=2, space="PSUM") as pp,
                tc.tile_pool(name=f"pap{rep}", bufs=2, space="PSUM") as pap,
            ):
                def warmup():
                    # hold the PE p-state up while the first DMAs land
                    nc.gpsimd.memset(warm_t[:], 0.0)
                    for i in range(40):
                        pw = pqk.tile([128, 128], F32, name=f"warm{i}",
                                      tag="pqk")
                        nc.tensor.matmul(pw[:], warm_t[:], warm_t[:],
                                         start=True, stop=True)
                        yield

                def phase1_window(w):
                    xtw = xtp.tile([128, NCI, W], BF16, name="xtw", tag="xtw")
                    if w == 0:
                        # HWDGE costs ~0.6us flat per DMA: few, mid-size
                        # chunks ordered so the first chains start early
                        nc.sync.dma_start(wqt[:, 0:4, :], wq_in[:, 0:4, :])
                        nc.sync.dma_start(xtw[:, 0:2, :], xt_in[:, w, 0:2, :])
                        nc.sync.dma_start(wqt[:, 4:, :], wq_in[:, 4:, :])
                        nc.sync.dma_start(xtw[:, 2:4, :], xt_in[:, w, 2:4, :])
                        nc.sync.dma_start(wkt[:], wk_in[:])
                        nc.sync.dma_start(xtw[:, 4:6, :], xt_in[:, w, 4:6, :])
                        nc.sync.dma_start(wvt[:], wv_in[:])
                        nc.sync.dma_start(xtw[:, 6:, :], xt_in[:, w, 6:, :])
                        nc.sync.dma_start(vones_t[:], vones_in[:])
                    elif w == 1:
                        nc.sync.dma_start(xtw[:, 0:4, :], xt_in[:, w, 0:4, :])
                        nc.sync.dma_start(xtw[:, 4:, :], xt_in[:, w, 4:, :])
                        nc.sync.dma_start(tri_t[:], tri_in[:])
                    else:
                        nc.sync.dma_start(xtw[:, 0:4, :], xt_in[:, w, 0:4, :])
                        nc.sync.dma_start(xtw[:, 4:, :], xt_in[:, w, 4:, :])
                    if w == 5:
                        nc.sync.dma_start(w3t[:], w3_in[:])

                    for fam, wt, dst in (("q", wqt, qt_tiles),
                                         ("k", wkt, kt_tiles)):
                        ps = pqk.tile([128, W], F32, name=f"ps_{fam}{w}",
                                      tag="pqk")
                        for ci in range(NCI):
                            nc.tensor.matmul(ps[:], wt[:, ci, :],
                                             xtw[:, ci, :],
                                             start=(ci == 0),
                                             stop=(ci == NCI - 1))
                            if ci % 4 == 3:
                                yield
                        sb = (qtp if fam == "q" else ktp).tile(
                            [128, W], BF16, name=f"{fam}t{w}", tag=f"{fam}t")
                        nc.vector.tensor_copy(sb[:], ps[:])
                        dst[w] = sb

                    # V directly in [k-token, head*dim] orientation:
                    # lhsT = x^T 128-token slice, rhs = wv chunk.
                    vw = vp.tile([128, 4, 130], BF16, name=f"vw{w}", tag="vw")
                    vwr = vw.rearrange("p k (l e) -> p k l e", e=65)
                    for kcl in range(4):
                        psv = pqk.tile([128, W], F32, name=f"psv{w}_{kcl}",
                                       tag="pqk")
                        for ci in range(NCI):
                            nc.tensor.matmul(
                                psv[:, 0:128],
                                xtw[:, ci, KC * kcl:KC * (kcl + 1)],
                                wvt[:, ci, :],
                                start=(ci == 0), stop=(ci == NCI - 1))
                        # both heads in one strided copy (PSUM: DVE only)
                        nc.vector.tensor_copy(vwr[:, kcl, :, 0:64],
                                              psv[:, 0:128].rearrange(
                                                  "p (l e) -> p l e", e=64))
                        yield
                    nc.gpsimd.tensor_copy(vwr[:, :, :, 64], vones_t[:])
                    v_tiles[w] = vw

                    aw = attp.tile([128, W], BF16, name=f"attw{w}", tag="attw")
                    att_tiles[w] = aw

                def emit_tail(tb, tj, tqw, tpsas):
                    preps = []
                    for l in range(HPC):
                        rc = rcpool.tile([1, W], F32R,
                                         name=f"rc{tb}{tj}{l}", tag="rc")
                        with nc.allow_low_precision(reason="f32r recip"):
                            nc.vector.reciprocal(rc[:], tpsas[l][64:65, :])
                        prep = rcpool.tile([64, W], F32R,
                                           name=f"prs{tb}{tj}{l}",
                                           tag=f"prs{l}")
                        nc.gpsimd.partition_broadcast(prep[:], rc[:])
                        preps.append(prep)
                    if KDBG == "prep":
                        for l in range(HPC):
                            nc.vector.tensor_copy(
                                dbgkeep[tqw][64 * l:64 * l + 64, :],
                                preps[l][:])
                    if KDBG == "rs":
                        for l in range(HPC):
                            nc.vector.tensor_copy(
                                dbgkeep[tqw][32 * l:32 * l + 1, :],
                                tpsas[l][64:65, :])
                    for l in range(HPC):
                        nc.vector.scalar_tensor_tensor(
                            att_tiles[tqw][64 * l:64 * l + 64, :],
                            tpsas[l][0:64, :], 1.0,
                            preps[l][:],
                            MULT, MULT)
                    if tb == 0:
                        for h in range(2):
                            nc.sync.dma_start(
                                cc_in[0][2 * tj + h, :, :],
                                att_tiles[tqw][:, HQS * h:HQS * (h + 1)])
                    else:
                        g = 1 + tj // 2
                        jj = tj % 2
                        for q in range(4):
                            nc.sync.dma_start(
                                cc_in[g][4 * jj + q, :, :],
                                att_tiles[tqw][:, QTR * q:QTR * (q + 1)])

                def collective(g):
                    if not knocc:
                        nc.gpsimd.collective_compute(
                            "AllToAll", mybir.AluOpType.bypass,
                            ins=[cc_in[g].opt()], outs=[cc_out[g].opt()],
                            replica_groups=[list(range(NCORES))])

                state = {"pending": None}
                att7_anchors = []

                def attention_window(qw, post_tail=None):
                    """Yields one unit per (pair, head). Chunk pair order:
                    off-diagonal ascending, then diagonal pairs in
                    DESCENDING kcl so the first (full-width) matmul
                    initializes the psas accumulator region."""
                    b, j = qw // WPB, qw % WPB
                    nk = 4 * (j + 1)
                    # pair list: (kca, kcb, widths) in processing order
                    pairs = [(2 * k2, 2 * k2 + 1, W, W)
                             for k2 in range(2 * j)]
                    # diagonal: kcl 3,2 then 1,0 (prefix widths 128*(kcl+1))
                    pairs.append((4 * j + 3, 4 * j + 2, W, 384))
                    pairs.append((4 * j + 1, 4 * j + 0, 256, 128))
                    npairs = len(pairs)
                    psas = [pap.tile([65, W], F32, name=f"psa{b}{l}{j}",
                                     tag="psa") for l in range(HPC)]
                    for pi, (kca, kcb, wa, wb) in enumerate(pairs):
                        diag = kca >= 4 * j
                        for l in range(HPC):
                            pss = pp.tile([128, 2, W], F32,
                                          name=f"pss{b}{l}{j}{pi}",
                                          tag="pp2")
                            for half, kc, wd in ((0, kca, wa), (1, kcb, wb)):
                                kw = WPB * b + kc // 4
                                kcl = kc % 4
                                nc.tensor.matmul(
                                    pss[:, half, 0:wd],
                                    kt_tiles[kw][64 * l:64 * l + 64,
                                                 KC * kcl:KC * (kcl + 1)],
                                    qt_tiles[qw][64 * l:64 * l + 64, 0:wd],
                                    start=True, stop=True)
                            pb = pbp.tile([128, 2, W], BF16,
                                          name=f"pb{b}{l}{j}{pi}",
                                          tag="pb")
                            if not diag:
                                nc.scalar.activation(
                                    pb.rearrange("p h f -> p (h f)"),
                                    pss.rearrange("p h f -> p (h f)"),
                                    EXP)
                            else:
                                for half, kc, wd in ((0, kca, wa),
                                                     (1, kcb, wb)):
                                    nc.scalar.activation(
                                        pb[:, half, 0:wd],
                                        pss[:, half, 0:wd], EXP)
                                    # upper-tri mask on the 128-col
                                    # boundary block of this chunk
                                    kcl = kc % 4
                                    blk = pb[:, half,
                                             KC * kcl:KC * (kcl + 1)]
                                    nc.vector.tensor_mul(blk, blk, tri_t[:])
                            first = pi == 0
                            last = pi == npairs - 1
                            for half, kc, wd in ((0, kca, wa), (1, kcb, wb)):
                                kw = WPB * b + kc // 4
                                kcl = kc % 4
                                mm = nc.tensor.matmul(
                                    psas[l][:, 0:wd],
                                    v_tiles[kw][:, kcl,
                                                65 * l:65 * l + 65],
                                    pb[:, half, 0:wd],
                                    start=(first and half == 0),
                                    stop=(last and half == 1))
                            if qw == NW - 1:
                                att7_anchors.append(mm)
                            yield
                        if pi == 0 and state["pending"] is not None:
                            emit_tail(*state["pending"])
                            state["pending"] = None
                            if post_tail is not None:
                                post_tail()
                    state["pending"] = (b, j, qw, psas)

                def anchor(inst, idx, why):
                    if att7_anchors:
                        a = att7_anchors[min(idx, len(att7_anchors) - 1)]
                        bass._add_dep_helper(inst.ins, a.ins, sync=False,
                                             reason=why)

                def phase3_loads(rt_tiles, g, anchor0=None):
                    rt_src = cc_in[g] if knocc else cc_out[g]
                    for s in range(NCORES):
                        rts = finp.tile([128, GCW[g]], BF16,
                                        name=f"rt{g}_{s}", tag="rt")
                        ld = nc.sync.dma_start(rts[:], rt_src[s, :, :])
                        if anchor0 is not None:
                            anchor(ld, anchor0 + s // 2,
                                   "late rt load placement")
                        rt_tiles.append(rts)
                        yield

                def phase3_units(rt_tiles, out_r, g, use_act, anchor0=None):
                    co, cw = GCOL[g], GCW[g]
                    for ec in range(NCI):
                        pso = pqk.tile([128, cw], F32,
                                       name=f"pso{g}_{ec}", tag="pqk")
                        for s in range(NCORES):
                            mm = nc.tensor.matmul(
                                pso[:],
                                w3t[:, s, KC * ec:KC * (ec + 1)],
                                rt_tiles[s],
                                start=(s == 0), stop=(s == NCORES - 1))
                            if anchor0 is not None and s == 0:
                                anchor(mm, anchor0 + ec, "ph3a interleave")
                        ot = otp.tile([128, cw], F32,
                                      name=f"ot{g}_{ec}", tag="ot")
                        if use_act and ec % 2 == 1:
                            nc.scalar.copy(ot[:], pso[:])
                        else:
                            nc.vector.tensor_copy(ot[:], pso[:])
                        nc.sync.dma_start(out_r[ec][:, co:co + cw], ot[:])
                        yield

                def drain(gen):
                    for _ in gen:
                        pass

                def chain(*gens):
                    for g in gens:
                        yield from g

                def imerge(primary, fill, n_primary, n_fill):
                    """Advance `fill` n_fill units spread across primary's
                    units; drains primary, leaves extra fill for later."""
                    done_f = 0
                    i = 0
                    for _ in primary:
                        i += 1
                        want = min(n_fill, (i * n_fill + n_primary - 1)
                                   // n_primary)
                        while done_f < want:
                            if next(fill, StopIteration) is StopIteration:
                                done_f = n_fill
                                break
                            done_f += 1
                    while done_f < n_fill:
                        if next(fill, StopIteration) is StopIteration:
                            break
                        done_f += 1

                def att_units(qw):
                    j = qw % WPB
                    return (2 * j + 2) * HPC

                # ---------- schedule ----------
                out_r = out_t.ap().rearrange("(e p) f -> e p f", p=128)
                rt_a, rt_b = [], []
                drain(warmup())
                drain(phase1_window(0))
                drain(phase1_window(1))
                # fill chain: remaining phase-1 windows. NOTHING that
                # waits on a collective may be interleaved into the
                # attention stream: engine SEQs evaluate waits in-order,
                # so one waiting DMA (or matmul) stalls everything
                # emitted after it on that engine.
                fills = chain(
                    phase1_window(2), phase1_window(3), phase1_window(4),
                    phase1_window(5), phase1_window(6), phase1_window(7),
                )
                budgets = [8, 8, 8, 8, 2, 6, 8, 8]
                posts = {WPB: lambda: collective(0),
                         WPB + 2: lambda: collective(1)}
                for qw in range(NW):
                    imerge(attention_window(qw, post_tail=posts.get(qw)),
                           fills, att_units(qw), budgets[qw])
                drain(fills)
                if state["pending"] is not None:
                    emit_tail(*state["pending"])
                    state["pending"] = None
                collective(2)

                # ---------- debug dumps ----------
                if KDBG:
                    out_r = out_t.ap().rearrange("(e p) f -> e p f", p=128)
                    if KDBG in ("prep", "rs"):
                        for w in range(NW):
                            nc.sync.dma_start(out_r[w], dbgkeep[w][:])
                    else:
                        dbg_src = {"q": qt_tiles, "k": kt_tiles,
                                   "att": att_tiles, "v": v_tiles}[KDBG]
                        with tc.tile_pool(name=f"dbgp{rep}", bufs=2) as dbgp:
                            for w in range(NW):
                                st = dbgp.tile([128, W], F32, name=f"dbg{w}",
                                               tag="dbg")
                                if KDBG == "v":
                                    nc.vector.tensor_copy(
                                        st[:], dbg_src[w][:].rearrange(
                                            "p a b -> p (a b)")[:, 0:W])
                                else:
                                    nc.vector.tensor_copy(st[:], dbg_src[w][:])
                                nc.sync.dma_start(out_r[w], st[:])
                        continue

                if not KDBG:
                    # phase3a: anchored into attention 7's exp-bound tail
                    # (collective 0 landed long before). Anchors both stop
                    # the scheduler from hoisting the loads so early that
                    # their collective wait blocks the SP DMA queue, and
                    # let the pso matmuls fill attention-7 exp bubbles.
                    drain(phase3_loads(rt_a, 0, anchor0=0))
                    drain(phase3_units(rt_a, out_r, 0, use_act=True,
                                       anchor0=8))
                    # phase3 b1/b2 wait on late collectives: anchored after
                    # attention 7's last unit (relative, so KREP-unrolled
                    # reps each keep their own tail in place)
                    rt_b2 = []
                    drain(phase3_loads(rt_b, 1, anchor0=99))
                    drain(phase3_units(rt_b, out_r, 1, use_act=True,
                                       anchor0=99))
                    drain(phase3_loads(rt_b2, 2, anchor0=99))
                    drain(phase3_units(rt_b2, out_r, 2, use_act=True,
                                       anchor0=99))

    nc.compile()
    _NC_CACHE[key] = nc
    return nc


def prep_inputs(x, wq, wk, wv, wp1, wp2):
    """Host-side sharding / layout prep. Returns per-core input dicts."""
    bf16 = mybir.dt.np(BF16)
    x = np.asarray(x, np.float32)
    wq = np.asarray(wq, np.float32)
    wk = np.asarray(wk, np.float32)
    wv = np.asarray(wv, np.float32)
    wp1 = np.asarray(wp1, np.float32)
    wp2 = np.asarray(wp2, np.float32)

    # x^T in [p, w, ci, f] layout; token order REVERSED inside each window
    xtf = x.reshape(TT, C).T                      # [C, TT]
    xt4 = xtf.reshape(NCI, 128, NW, W)[:, :, :, ::-1]
    xt_host = np.ascontiguousarray(
        xt4.transpose(1, 2, 0, 3)).astype(bf16)

    scale = 1.0 / np.sqrt(D)

    def wlay(wm):                                  # [C, 128] -> [128, NCI, 128]
        return np.ascontiguousarray(
            wm.reshape(NCI, 128, 128).transpose(1, 0, 2)).astype(bf16)

    w3 = wp1.reshape(C, C) @ wp2                   # [hd, e]
    w3_host = np.ascontiguousarray(
        w3.reshape(NCORES, 128, C).transpose(1, 0, 2)).astype(bf16)

    # keep where q'(col) <= k'(partition)
    p = np.arange(128)[:, None]
    f = np.arange(KC)[None, :]
    tri_host = np.ascontiguousarray((f <= p).astype(np.float32)).astype(bf16)
    vones_host = np.ones((128, 4, 2), np.float32).astype(bf16)

    in_maps = []
    for c in range(NCORES):
        h0 = HPC * c
        wq_c = wlay(wq[:, h0:h0 + HPC, :].reshape(C, HPC * D) * scale)
        wk_c = wlay(wk[:, h0:h0 + HPC, :].reshape(C, HPC * D))
        wv_c = wlay(wv[:, h0:h0 + HPC, :].reshape(C, HPC * D))
        in_maps.append({
            "xt": xt_host, "wq": wq_c, "wk": wk_c, "wv": wv_c,
            "w3": w3_host, "tri": tri_host,
            "vones": vones_host,
        })
    return in_maps


def assemble_output(results):
    # Collective group g gives core r a cw-column slice of window
    # j_base + r // (W//cw), slot q = r % (W//cw), att cols
    # [cw*q, cw*(q+1)) -- columns in reversed token order.
    parts = [(0, 0, HQS, 0),            # (batch, j_base, cw, out_t col0)
             (1, 0, QTR, HQS),
             (1, 2, QTR, HQS + QTR)]
    out = np.empty((TT, C), np.float32)
    for r in range(NCORES):
        ot = results[r]["out_t"]
        for b, j_base, cw, co in parts:
            spw = W // cw
            j = j_base + r // spw
            q = r % spw
            g0 = T * b + W * j + W - cw * (q + 1)
            out[g0:g0 + cw, :] = ot[:, co:co + cw][:, ::-1].T
    return out.reshape(B, T, C)


def kernel(x, wq, wk, wv, wp1, wp2):
    in_maps = prep_inputs(x, wq, wk, wv, wp1, wp2)
    nc = build_nc()
    res = run_bass_kernel_spmd(nc, in_maps, list(range(NCORES)))
    return assemble_output(res.results)


# revision 63
# speedup vs baseline: 1.1250x; 1.1250x over previous
"""Causal self-attention block (QKV -> causal attention -> 2 projections)
distributed over 8 NeuronCores via Bass/Tile.

Sharding: tensor-parallel over heads (2 heads/core, both batches on every
core). The whole on-device computation runs in transposed activation space
(channel/head-dim on partitions, tokens on the free axis) so no on-device
transposes of activations are ever needed.

Token order within each 512-token window is REVERSED (host-side, in x^T).
In reversed coordinates the causal mask becomes "q' <= k'", so every
diagonal-window narrowing is PREFIX-aligned:
  - scores for diagonal k-chunk kcl only need q' columns [0, 128*(kcl+1)),
    written at the PSUM bank start (PE PSUM writes must start banks);
  - the PV accumulation is narrowed the same way (diagonal chunks processed
    in DESCENDING kcl order so the first, full-width matmul initializes the
    accumulator region);
  - only one 128x128 upper-triangular block per diagonal chunk needs
    masking (done with one strided tensor_mul per chunk pair on DVE).

phase 1: Q^T, K^T = w^T @ x^T; V in [token, dim] orientation with an
         appended ones column (softmax row-sums for free in phase 2).
phase 2: S^T = K Q^T per 128k x 512q chunk, P = exp(S^T) on ScalarE,
         att^T accumulated on PE, normalized by the row-sum reciprocal
         (DVE recip -> Pool partition_broadcast -> DVE stt).
A2A:     two 8-rank AllToAlls (fp16) move att^T head-blocks so each core
         ends up owning 512 token rows of all 16 heads: one per batch.
         The batch-0 exchange is issued as soon as batch 0's attention
         is done and hides under batch-1 compute; only the batch-1
         exchange is tail-exposed.
phase 3: out^T = W3^T att^T with W3 = (wp1 . wp2) folded on the host;
         ec-outer loops per A2A group; group 0's matmuls are anchored
         into attention-7's exp-bound tail.

Work is emitted through a fine-grained unit interleave (attention window w
merged with phase-1 window w+2) so PE never waits on ScalarE exp; a few
warm-up matmuls hold the PE p-state up while the first DMAs land. Nothing
that waits on a collective may be scheduled inside the attention stream
(engine SEQs evaluate waits in-order), so phase-3 work is pinned late via
explicit no-sync anchor deps on attention-7 units.
"""

import os

import numpy as np

import concourse.bass as bass
import concourse.tile as tile
from concourse import bacc, mybir
from concourse.bass_utils import run_bass_kernel_spmd

B, T, C, H, D = 2, 2048, 1024, 16, 64
NCORES = 8
HPC = H // NCORES          # heads per core = 2
TT = B * T                 # 4096 flat (b, t) rows
W = 512                    # token window / q chunk
NW = TT // W               # 8 windows
WPB = T // W               # 4 windows per batch
QS = TT // NCORES          # 512 rows per core after A2A
HQS = QS // 2              # 256-token half window = per-batch A2A payload
QTR = QS // 4              # (retained for the host-side assemble map)
NG = 2                     # collective groups: {batch 0}, {batch 1}
GCW = (HQS, HQS)           # slot cols per collective group
GCOL = (0, HQS)            # out_t col offset per group
NCI = C // 128             # 8 channel chunks
KC = 128                   # k chunk
F32 = mybir.dt.float32
F32R = mybir.dt.float32r
BF16 = mybir.dt.float16
EXP = mybir.ActivationFunctionType.Exp
MULT = mybir.AluOpType.mult

_NC_CACHE = {}

KNOCC = int(os.environ.get("KNOCC", "0"))
KREP = int(os.environ.get("KREP", "1"))
KDBG = os.environ.get("KDBG", "")


def build_nc(krep=None, knocc=None):
    krep = KREP if krep is None else krep
    knocc = KNOCC if knocc is None else knocc
    key = (krep, knocc)
    if key in _NC_CACHE:
        return _NC_CACHE[key]
    nc = bacc.Bacc("TRN2", target_bir_lowering=False, debug=False,
                   num_devices=NCORES)
    xt_in = nc.dram_tensor("xt", [128, NW, NCI, W], BF16, kind="ExternalInput")
    wq_in = nc.dram_tensor("wq", [128, NCI, 128], BF16, kind="ExternalInput")
    wk_in = nc.dram_tensor("wk", [128, NCI, 128], BF16, kind="ExternalInput")
    wv_in = nc.dram_tensor("wv", [128, NCI, 128], BF16, kind="ExternalInput")
    w3_in = nc.dram_tensor("w3", [128, NCORES, C], BF16, kind="ExternalInput")
    tri_in = nc.dram_tensor("tri", [128, KC], BF16, kind="ExternalInput")
    vones_in = nc.dram_tensor("vones", [128, 4, 2], BF16, kind="ExternalInput")
    out_t = nc.dram_tensor("out_t", [C, QS], F32, kind="ExternalOutput")

    with tile.TileContext(nc) as tc:
      for rep in range(krep):
        with (
            tc.tile_pool(name=f"dramp{rep}", bufs=1, space="DRAM") as dramp,
            tc.tile_pool(name=f"dramq{rep}", bufs=1, space="DRAM") as dramq,
            tc.tile_pool(name=f"misc{rep}", bufs=1) as misc,
            tc.tile_pool(name=f"rcpool{rep}", bufs=4) as rcpool,
            tc.tile_pool(name=f"qtp{rep}", bufs=NW) as qtp,
            tc.tile_pool(name=f"ktp{rep}", bufs=NW) as ktp,
            tc.tile_pool(name=f"vp{rep}", bufs=NW) as vp,
            tc.tile_pool(name=f"attp{rep}", bufs=NW) as attp,
            tc.tile_pool(name=f"pbp{rep}", bufs=4) as pbp,
            tc.tile_pool(name=f"wqkv{rep}", bufs=1) as wqkv,
            tc.tile_pool(name=f"xtp{rep}", bufs=4) as xtp,
            tc.tile_pool(name=f"wpp{rep}", bufs=1) as wpp,
            tc.tile_pool(name=f"finp{rep}", bufs=NCORES) as finp,
            tc.tile_pool(name=f"otp{rep}", bufs=4) as otp,
            ):
            # Per-batch A2A bounce buffers: slot r = (window r//2, half
            # r%2), 256 cols. Group 0 (batch 0) is issued as soon as
            # batch 0's attention finishes and hides under batch-1
            # compute; only group 1 is tail-exposed. (Real HW collectives
            # measure ~free; fewer groups = fewer DMA issues.)
            pools = (dramp, dramq)
            cc_in = [pools[g].tile([NCORES, 128, GCW[g]], BF16,
                                   name=f"cin{g}", tag=f"cin{g}")
                     for g in range(NG)]
            cc_out = [pools[g].tile([NCORES, 128, GCW[g]], BF16,
                                    name=f"cout{g}", tag=f"cout{g}")
                      for g in range(NG)]

            tri_t = misc.tile([128, KC], BF16, name="tri_t")
            vones_t = misc.tile([128, 4, 2], BF16, name="vones_t")
            warm_t = misc.tile([128, 128], BF16, name="warm_t")
            wqt = wqkv.tile([128, NCI, 128], BF16, name="wqt")
            wkt = wqkv.tile([128, NCI, 128], BF16, name="wkt")
            wvt = wqkv.tile([128, NCI, 128], BF16, name="wvt")
            w3t = wpp.tile([128, NCORES, C], BF16, name="w3t")

            qt_tiles, kt_tiles, v_tiles, att_tiles = {}, {}, {}, {}

            dbgkeep = []
            if KDBG in ("prep", "rs"):
                dbgkeep = [misc.tile([128, W], F32, name=f"dk{i}")
                           for i in range(NW)]

            # phase 1 + phase 2 share one PSUM budget (8 banks):
            # qkv/ph3 ring 2 x 1 + scores 2 x 2 + psas 2 x 1 = 8
            with (
                tc.tile_pool(name=f"pqk{rep}", bufs=2, space="PSUM") as pqk,
                tc.tile_pool(name=f"pp{rep}", bufs=2, space="PSUM") as pp,
                tc.tile_pool(name=f"pap{rep}", bufs=2, space="PSUM") as pap,
            ):
                def warmup():
                    # hold the PE p-state up while the first DMAs land
                    nc.gpsimd.memset(warm_t[:], 0.0)
                    for i in range(40):
                        pw = pqk.tile([128, 128], F32, name=f"warm{i}",
                                      tag="pqk")
                        nc.tensor.matmul(pw[:], warm_t[:], warm_t[:],
                                         start=True, stop=True)
                        yield

                def phase1_window(w):
                    xtw = xtp.tile([128, NCI, W], BF16, name="xtw", tag="xtw")
                    if w == 0:
                        # HWDGE costs ~0.6us flat per DMA: few, mid-size
                        # chunks ordered so the first chains start early
                        nc.sync.dma_start(wqt[:, 0:4, :], wq_in[:, 0:4, :])
                        nc.sync.dma_start(xtw[:, 0:2, :], xt_in[:, w, 0:2, :])
                        nc.sync.dma_start(wqt[:, 4:, :], wq_in[:, 4:, :])
                        nc.sync.dma_start(xtw[:, 2:4, :], xt_in[:, w, 2:4, :])
                        nc.sync.dma_start(wkt[:], wk_in[:])
                        nc.sync.dma_start(xtw[:, 4:6, :], xt_in[:, w, 4:6, :])
                        nc.sync.dma_start(wvt[:], wv_in[:])
                        nc.sync.dma_start(xtw[:, 6:, :], xt_in[:, w, 6:, :])
                        nc.sync.dma_start(vones_t[:], vones_in[:])
                    elif w == 1:
                        nc.sync.dma_start(xtw[:, 0:4, :], xt_in[:, w, 0:4, :])
                        nc.sync.dma_start(xtw[:, 4:, :], xt_in[:, w, 4:, :])
                        nc.sync.dma_start(tri_t[:], tri_in[:])
                    else:
                        nc.sync.dma_start(xtw[:, 0:4, :], xt_in[:, w, 0:4, :])
                        nc.sync.dma_start(xtw[:, 4:, :], xt_in[:, w, 4:, :])
                    if w == 5:
                        nc.sync.dma_start(w3t[:], w3_in[:])

                    for fam, wt, dst in (("q", wqt, qt_tiles),
                                         ("k", wkt, kt_tiles)):
                        ps = pqk.tile([128, W], F32, name=f"ps_{fam}{w}",
                                      tag="pqk")
                        for ci in range(NCI):
                            nc.tensor.matmul(ps[:], wt[:, ci, :],
                                             xtw[:, ci, :],
                                             start=(ci == 0),
                                             stop=(ci == NCI - 1))
                            if ci % 4 == 3:
                                yield
                        sb = (qtp if fam == "q" else ktp).tile(
                            [128, W], BF16, name=f"{fam}t{w}", tag=f"{fam}t")
                        nc.vector.tensor_copy(sb[:], ps[:])
                        dst[w] = sb

                    # V directly in [k-token, head*dim] orientation:
                    # lhsT = x^T 128-token slice, rhs = wv chunk.
                    vw = vp.tile([128, 4, 130], BF16, name=f"vw{w}", tag="vw")
                    vwr = vw.rearrange("p k (l e) -> p k l e", e=65)
                    for kcl in range(4):
                        psv = pqk.tile([128, W], F32, name=f"psv{w}_{kcl}",
                                       tag="pqk")
                        for ci in range(NCI):
                            nc.tensor.matmul(
                                psv[:, 0:128],
                                xtw[:, ci, KC * kcl:KC * (kcl + 1)],
                                wvt[:, ci, :],
                                start=(ci == 0), stop=(ci == NCI - 1))
                        # both heads in one strided copy (PSUM: DVE only)
                        nc.vector.tensor_copy(vwr[:, kcl, :, 0:64],
                                              psv[:, 0:128].rearrange(
                                                  "p (l e) -> p l e", e=64))
                        yield
                    nc.gpsimd.tensor_copy(vwr[:, :, :, 64], vones_t[:])
                    v_tiles[w] = vw

                    aw = attp.tile([128, W], BF16, name=f"attw{w}", tag="attw")
                    att_tiles[w] = aw

                def emit_tail(tb, tj, tqw, tpsas):
                    preps = []
                    for l in range(HPC):
                        rc = rcpool.tile([1, W], F32R,
                                         name=f"rc{tb}{tj}{l}", tag="rc")
                        with nc.allow_low_precision(reason="f32r recip"):
                            nc.vector.reciprocal(rc[:], tpsas[l][64:65, :])
                        prep = rcpool.tile([64, W], F32R,
                                           name=f"prs{tb}{tj}{l}",
                                           tag=f"prs{l}")
                        nc.gpsimd.partition_broadcast(prep[:], rc[:])
                        preps.append(prep)
                    if KDBG == "prep":
                        for l in range(HPC):
                            nc.vector.tensor_copy(
                                dbgkeep[tqw][64 * l:64 * l + 64, :],
                                preps[l][:])
                    if KDBG == "rs":
                        for l in range(HPC):
                            nc.vector.tensor_copy(
                                dbgkeep[tqw][32 * l:32 * l + 1, :],
                                tpsas[l][64:65, :])
                    for l in range(HPC):
                        nc.vector.scalar_tensor_tensor(
                            att_tiles[tqw][64 * l:64 * l + 64, :],
                            tpsas[l][0:64, :], 1.0,
                            preps[l][:],
                            MULT, MULT)
                    for h in range(2):
                        nc.sync.dma_start(
                            cc_in[tb][2 * tj + h, :, :],
                            att_tiles[tqw][:, HQS * h:HQS * (h + 1)])

                def collective(g):
                    if not knocc:
                        nc.gpsimd.collective_compute(
                            "AllToAll", mybir.AluOpType.bypass,
                            ins=[cc_in[g].opt()], outs=[cc_out[g].opt()],
                            replica_groups=[list(range(NCORES))])

                state = {"pending": None}
                att7_anchors = []

                def attention_window(qw, post_tail=None):
                    """Yields one unit per (pair, head). Chunk pair order:
                    off-diagonal ascending, then diagonal pairs in
                    DESCENDING kcl so the first (full-width) matmul
                    initializes the psas accumulator region."""
                    b, j = qw // WPB, qw % WPB
                    nk = 4 * (j + 1)
                    # pair list: (kca, kcb, widths) in processing order
                    pairs = [(2 * k2, 2 * k2 + 1, W, W)
                             for k2 in range(2 * j)]
                    # diagonal: kcl 3,2 then 1,0 (prefix widths 128*(kcl+1))
                    pairs.append((4 * j + 3, 4 * j + 2, W, 384))
                    pairs.append((4 * j + 1, 4 * j + 0, 256, 128))
                    npairs = len(pairs)
                    psas = [pap.tile([65, W], F32, name=f"psa{b}{l}{j}",
                                     tag="psa") for l in range(HPC)]
                    for pi, (kca, kcb, wa, wb) in enumerate(pairs):
                        diag = kca >= 4 * j
                        for l in range(HPC):
                            pss = pp.tile([128, 2, W], F32,
                                          name=f"pss{b}{l}{j}{pi}",
                                          tag="pp2")
                            for half, kc, wd in ((0, kca, wa), (1, kcb, wb)):
                                kw = WPB * b + kc // 4
                                kcl = kc % 4
                                nc.tensor.matmul(
                                    pss[:, half, 0:wd],
                                    kt_tiles[kw][64 * l:64 * l + 64,
                                                 KC * kcl:KC * (kcl + 1)],
                                    qt_tiles[qw][64 * l:64 * l + 64, 0:wd],
                                    start=True, stop=True)
                            pb = pbp.tile([128, 2, W], BF16,
                                          name=f"pb{b}{l}{j}{pi}",
                                          tag="pb")
                            if not diag:
                                nc.scalar.activation(
                                    pb.rearrange("p h f -> p (h f)"),
                                    pss.rearrange("p h f -> p (h f)"),
                                    EXP)
                            else:
                                for half, kc, wd in ((0, kca, wa),
                                                     (1, kcb, wb)):
                                    nc.scalar.activation(
                                        pb[:, half, 0:wd],
                                        pss[:, half, 0:wd], EXP)
                                    # upper-tri mask on the 128-col
                                    # boundary block of this chunk
                                    kcl = kc % 4
                                    blk = pb[:, half,
                                             KC * kcl:KC * (kcl + 1)]
                                    nc.vector.tensor_mul(blk, blk, tri_t[:])
                            first = pi == 0
                            last = pi == npairs - 1
                            for half, kc, wd in ((0, kca, wa), (1, kcb, wb)):
                                kw = WPB * b + kc // 4
                                kcl = kc % 4
                                mm = nc.tensor.matmul(
                                    psas[l][:, 0:wd],
                                    v_tiles[kw][:, kcl,
                                                65 * l:65 * l + 65],
                                    pb[:, half, 0:wd],
                                    start=(first and half == 0),
                                    stop=(last and half == 1))
                            if qw == NW - 1:
                                att7_anchors.append(mm)
                            yield
                        if pi == 0 and state["pending"] is not None:
                            emit_tail(*state["pending"])
                            state["pending"] = None
                            if post_tail is not None:
                                post_tail()
                    state["pending"] = (b, j, qw, psas)

                def anchor(inst, idx, why):
                    if att7_anchors:
                        a = att7_anchors[min(idx, len(att7_anchors) - 1)]
                        bass._add_dep_helper(inst.ins, a.ins, sync=False,
                                             reason=why)

                def phase3_loads(rt_tiles, g, anchor0=None):
                    rt_src = cc_in[g] if knocc else cc_out[g]
                    for s in range(NCORES):
                        rts = finp.tile([128, GCW[g]], BF16,
                                        name=f"rt{g}_{s}", tag="rt")
                        ld = nc.sync.dma_start(rts[:], rt_src[s, :, :])
                        if anchor0 is not None:
                            anchor(ld, anchor0 + s // 2,
                                   "late rt load placement")
                        rt_tiles.append(rts)
                        yield

                def phase3_units(rt_tiles, out_r, g, use_act, anchor0=None):
                    co, cw = GCOL[g], GCW[g]
                    for ec in range(NCI):
                        pso = pqk.tile([128, cw], F32,
                                       name=f"pso{g}_{ec}", tag="pqk")
                        for s in range(NCORES):
                            mm = nc.tensor.matmul(
                                pso[:],
                                w3t[:, s, KC * ec:KC * (ec + 1)],
                                rt_tiles[s],
                                start=(s == 0), stop=(s == NCORES - 1))
                            if anchor0 is not None and s == 0:
                                anchor(mm, anchor0 + ec, "ph3a interleave")
                        ot = otp.tile([128, cw], F32,
                                      name=f"ot{g}_{ec}", tag="ot")
                        if use_act and ec % 2 == 1:
                            nc.scalar.copy(ot[:], pso[:])
                        else:
                            nc.vector.tensor_copy(ot[:], pso[:])
                        nc.sync.dma_start(out_r[ec][:, co:co + cw], ot[:])
                        yield

                def drain(gen):
                    for _ in gen:
                        pass

                def chain(*gens):
                    for g in gens:
                        yield from g

                def imerge(primary, fill, n_primary, n_fill):
                    """Advance `fill` n_fill units spread across primary's
                    units; drains primary, leaves extra fill for later."""
                    done_f = 0
                    i = 0
                    for _ in primary:
                        i += 1
                        want = min(n_fill, (i * n_fill + n_primary - 1)
                                   // n_primary)
                        while done_f < want:
                            if next(fill, StopIteration) is StopIteration:
                                done_f = n_fill
                                break
                            done_f += 1
                    while done_f < n_fill:
                        if next(fill, StopIteration) is StopIteration:
                            break
                        done_f += 1

                def att_units(qw):
                    j = qw % WPB
                    return (2 * j + 2) * HPC

                # ---------- schedule ----------
                out_r = out_t.ap().rearrange("(e p) f -> e p f", p=128)
                rt_a, rt_b = [], []
                drain(warmup())
                drain(phase1_window(0))
                drain(phase1_window(1))
                # fill chain: remaining phase-1 windows. NOTHING that
                # waits on a collective may be interleaved into the
                # attention stream: engine SEQs evaluate waits in-order,
                # so one waiting DMA (or matmul) stalls everything
                # emitted after it on that engine.
                fills = chain(
                    phase1_window(2), phase1_window(3), phase1_window(4),
                    phase1_window(5), phase1_window(6), phase1_window(7),
                )
                budgets = [8, 8, 8, 8, 2, 6, 8, 8]
                posts = {WPB: lambda: collective(0)}
                for qw in range(NW):
                    imerge(attention_window(qw, post_tail=posts.get(qw)),
                           fills, att_units(qw), budgets[qw])
                drain(fills)
                if state["pending"] is not None:
                    emit_tail(*state["pending"])
                    state["pending"] = None
                collective(1)

                # ---------- debug dumps ----------
                if KDBG:
                    out_r = out_t.ap().rearrange("(e p) f -> e p f", p=128)
                    if KDBG in ("prep", "rs"):
                        for w in range(NW):
                            nc.sync.dma_start(out_r[w], dbgkeep[w][:])
                    else:
                        dbg_src = {"q": qt_tiles, "k": kt_tiles,
                                   "att": att_tiles, "v": v_tiles}[KDBG]
                        with tc.tile_pool(name=f"dbgp{rep}", bufs=2) as dbgp:
                            for w in range(NW):
                                st = dbgp.tile([128, W], F32, name=f"dbg{w}",
                                               tag="dbg")
                                if KDBG == "v":
                                    nc.vector.tensor_copy(
                                        st[:], dbg_src[w][:].rearrange(
                                            "p a b -> p (a b)")[:, 0:W])
                                else:
                                    nc.vector.tensor_copy(st[:], dbg_src[w][:])
                                nc.sync.dma_start(out_r[w], st[:])
                        continue

                if not KDBG:
                    # phase3a: anchored into attention 7's exp-bound tail
                    # (collective 0 landed long before). Anchors both stop
                    # the scheduler from hoisting the loads so early that
                    # their collective wait blocks the SP DMA queue, and
                    # let the pso matmuls fill attention-7 exp bubbles.
                    drain(phase3_loads(rt_a, 0, anchor0=0))
                    drain(phase3_units(rt_a, out_r, 0, use_act=True,
                                       anchor0=8))
                    # phase3b waits on the last collective: anchored after
                    # attention 7's last unit (relative, so KREP-unrolled
                    # reps each keep their own tail in place)
                    drain(phase3_loads(rt_b, 1, anchor0=99))
                    drain(phase3_units(rt_b, out_r, 1, use_act=True,
                                       anchor0=99))

    nc.compile()
    _NC_CACHE[key] = nc
    return nc


def prep_inputs(x, wq, wk, wv, wp1, wp2):
    """Host-side sharding / layout prep. Returns per-core input dicts."""
    bf16 = mybir.dt.np(BF16)
    x = np.asarray(x, np.float32)
    wq = np.asarray(wq, np.float32)
    wk = np.asarray(wk, np.float32)
    wv = np.asarray(wv, np.float32)
    wp1 = np.asarray(wp1, np.float32)
    wp2 = np.asarray(wp2, np.float32)

    # x^T in [p, w, ci, f] layout; token order REVERSED inside each window
    xtf = x.reshape(TT, C).T                      # [C, TT]
    xt4 = xtf.reshape(NCI, 128, NW, W)[:, :, :, ::-1]
    xt_host = np.ascontiguousarray(
        xt4.transpose(1, 2, 0, 3)).astype(bf16)

    scale = 1.0 / np.sqrt(D)

    def wlay(wm):                                  # [C, 128] -> [128, NCI, 128]
        return np.ascontiguousarray(
            wm.reshape(NCI, 128, 128).transpose(1, 0, 2)).astype(bf16)

    w3 = wp1.reshape(C, C) @ wp2                   # [hd, e]
    w3_host = np.ascontiguousarray(
        w3.reshape(NCORES, 128, C).transpose(1, 0, 2)).astype(bf16)

    # keep where q'(col) <= k'(partition)
    p = np.arange(128)[:, None]
    f = np.arange(KC)[None, :]
    tri_host = np.ascontiguousarray((f <= p).astype(np.float32)).astype(bf16)
    vones_host = np.ones((128, 4, 2), np.float32).astype(bf16)

    in_maps = []
    for c in range(NCORES):
        h0 = HPC * c
        wq_c = wlay(wq[:, h0:h0 + HPC, :].reshape(C, HPC * D) * scale)
        wk_c = wlay(wk[:, h0:h0 + HPC, :].reshape(C, HPC * D))
        wv_c = wlay(wv[:, h0:h0 + HPC, :].reshape(C, HPC * D))
        in_maps.append({
            "xt": xt_host, "wq": wq_c, "wk": wk_c, "wv": wv_c,
            "w3": w3_host, "tri": tri_host,
            "vones": vones_host,
        })
    return in_maps


def assemble_output(results):
    # Collective group g gives core r a cw-column slice of window
    # j_base + r // (W//cw), slot q = r % (W//cw), att cols
    # [cw*q, cw*(q+1)) -- columns in reversed token order.
    parts = [(0, 0, HQS, 0),            # (batch, j_base, cw, out_t col0)
             (1, 0, HQS, HQS)]
    out = np.empty((TT, C), np.float32)
    for r in range(NCORES):
        ot = results[r]["out_t"]
        for b, j_base, cw, co in parts:
            spw = W // cw
            j = j_base + r // spw
            q = r % spw
            g0 = T * b + W * j + W - cw * (q + 1)
            out[g0:g0 + cw, :] = ot[:, co:co + cw][:, ::-1].T
    return out.reshape(B, T, C)


def kernel(x, wq, wk, wv, wp1, wp2):
    in_maps = prep_inputs(x, wq, wk, wv, wp1, wp2)
    nc = build_nc()
    res = run_bass_kernel_spmd(nc, in_maps, list(range(NCORES)))
    return assemble_output(res.results)


# revision 70
# speedup vs baseline: 1.2441x; 1.1059x over previous
"""Causal self-attention block (QKV -> causal attention -> 2 projections)
distributed over 8 NeuronCores via Bass/Tile.

Sharding: tensor-parallel over heads (2 heads/core, both batches on every
core). The whole on-device computation runs in transposed activation space
(channel/head-dim on partitions, tokens on the free axis) so no on-device
transposes of activations are ever needed.

Token order within each 512-token window is REVERSED (host-side, in x^T).
In reversed coordinates the causal mask becomes "q' <= k'", so every
diagonal-window narrowing is PREFIX-aligned:
  - scores for diagonal k-chunk kcl only need q' columns [0, 128*(kcl+1)),
    written at the PSUM bank start (PE PSUM writes must start banks);
  - the PV accumulation is narrowed the same way (diagonal chunks processed
    in DESCENDING kcl order so the first, full-width matmul initializes the
    accumulator region);
  - only one 128x128 upper-triangular block per diagonal chunk needs
    masking (done with one strided tensor_mul per chunk pair on DVE).

phase 1: Q^T, K^T = w^T @ x^T; V in [token, dim] orientation with an
         appended ones column (softmax row-sums for free in phase 2).
phase 2: S^T = K Q^T per 128k x 512q chunk, P = exp(S^T) on ScalarE,
         att^T accumulated on PE, normalized by the row-sum reciprocal
         (DVE recip -> Pool partition_broadcast -> DVE stt).
A2A:     two 8-rank AllToAlls (fp16) move att^T head-blocks so each core
         ends up owning 512 token rows of all 16 heads: one per batch.
         The batch-0 exchange is issued as soon as batch 0's attention
         is done and hides under batch-1 compute; only the batch-1
         exchange is tail-exposed.
phase 3: out^T = W3^T att^T with W3 = (wp1 . wp2) folded on the host;
         ec-outer loops per A2A group; group 0's matmuls are anchored
         into attention-7's exp-bound tail.

Work is emitted through a fine-grained unit interleave (attention window w
merged with phase-1 window w+2) so PE never waits on ScalarE exp; a few
warm-up matmuls hold the PE p-state up while the first DMAs land. Nothing
that waits on a collective may be scheduled inside the attention stream
(engine SEQs evaluate waits in-order), so phase-3 work is pinned late via
explicit no-sync anchor deps on attention-7 units.
"""

import os

import numpy as np

import concourse.bass as bass
import concourse.tile as tile
from concourse import bacc, mybir
from concourse.bass_utils import run_bass_kernel_spmd

B, T, C, H, D = 2, 2048, 1024, 16, 64
NCORES = 8
HPC = H // NCORES          # heads per core = 2
TT = B * T                 # 4096 flat (b, t) rows
W = 512                    # token window / q chunk
NW = TT // W               # 8 windows
WPB = T // W               # 4 windows per batch
QS = TT // NCORES          # 512 rows per core after A2A
HQS = QS // 2              # 256-token half window = per-batch A2A payload
QTR = QS // 4              # (retained for the host-side assemble map)
NG = 2                     # collective groups: {batch 0}, {batch 1}
GCW = (HQS, HQS)           # slot cols per collective group
GCOL = (0, HQS)            # out_t col offset per group
NCI = C // 128             # 8 channel chunks
KC = 128                   # k chunk
F32 = mybir.dt.float32
F32R = mybir.dt.float32r
BF16 = mybir.dt.float16
EXP = mybir.ActivationFunctionType.Exp
MULT = mybir.AluOpType.mult

_NC_CACHE = {}

KNOCC = int(os.environ.get("KNOCC", "0"))
KREP = int(os.environ.get("KREP", "1"))
KDBG = os.environ.get("KDBG", "")


def build_nc(krep=None, knocc=None):
    krep = KREP if krep is None else krep
    knocc = KNOCC if knocc is None else knocc
    key = (krep, knocc)
    if key in _NC_CACHE:
        return _NC_CACHE[key]
    nc = bacc.Bacc("TRN2", target_bir_lowering=False, debug=False,
                   num_devices=NCORES)
    xt_in = nc.dram_tensor("xt", [128, NW, NCI, W], BF16, kind="ExternalInput")
    wq_in = nc.dram_tensor("wq", [128, NCI, 128], BF16, kind="ExternalInput")
    wk_in = nc.dram_tensor("wk", [128, NCI, 128], BF16, kind="ExternalInput")
    wv_in = nc.dram_tensor("wv", [128, NCI, 128], BF16, kind="ExternalInput")
    w3_in = nc.dram_tensor("w3", [128, NCORES, C], BF16, kind="ExternalInput")
    tri_in = nc.dram_tensor("tri", [128, KC], BF16, kind="ExternalInput")
    vones_in = nc.dram_tensor("vones", [128, 4, 2], BF16, kind="ExternalInput")
    out_t = nc.dram_tensor("out_t", [C, QS], F32, kind="ExternalOutput")

    with tile.TileContext(nc) as tc:
      for rep in range(krep):
        with (
            tc.tile_pool(name=f"dramp{rep}", bufs=1, space="DRAM") as dramp,
            tc.tile_pool(name=f"dramq{rep}", bufs=1, space="DRAM") as dramq,
            tc.tile_pool(name=f"misc{rep}", bufs=1) as misc,
            tc.tile_pool(name=f"rcpool{rep}", bufs=4) as rcpool,
            tc.tile_pool(name=f"qtp{rep}", bufs=NW) as qtp,
            tc.tile_pool(name=f"ktp{rep}", bufs=NW) as ktp,
            tc.tile_pool(name=f"vp{rep}", bufs=NW) as vp,
            tc.tile_pool(name=f"attp{rep}", bufs=NW) as attp,
            tc.tile_pool(name=f"pbp{rep}", bufs=4) as pbp,
            tc.tile_pool(name=f"wqkv{rep}", bufs=1) as wqkv,
            tc.tile_pool(name=f"xtp{rep}", bufs=4) as xtp,
            tc.tile_pool(name=f"wpp{rep}", bufs=1) as wpp,
            tc.tile_pool(name=f"finp{rep}", bufs=NCORES) as finp,
            tc.tile_pool(name=f"otp{rep}", bufs=4) as otp,
            ):
            # Per-batch A2A bounce buffers: slot r = (window r//2, half
            # r%2), 256 cols. Group 0 (batch 0) is issued as soon as
            # batch 0's attention finishes and hides under batch-1
            # compute; only group 1 is tail-exposed. (Real HW collectives
            # measure ~free; fewer groups = fewer DMA issues.)
            pools = (dramp, dramq)
            cc_in = [pools[g].tile([NCORES, 128, GCW[g]], BF16,
                                   name=f"cin{g}", tag=f"cin{g}")
                     for g in range(NG)]
            cc_out = [pools[g].tile([NCORES, 128, GCW[g]], BF16,
                                    name=f"cout{g}", tag=f"cout{g}")
                      for g in range(NG)]

            tri_t = misc.tile([128, KC], BF16, name="tri_t")
            vones_t = misc.tile([128, 4, 2], BF16, name="vones_t")
            warm_t = misc.tile([128, 128], BF16, name="warm_t")
            wqt = wqkv.tile([128, NCI, 128], BF16, name="wqt")
            wkt = wqkv.tile([128, NCI, 128], BF16, name="wkt")
            wvt = wqkv.tile([128, NCI, 128], BF16, name="wvt")
            w3t = wpp.tile([128, NCORES, C], BF16, name="w3t")

            qt_tiles, kt_tiles, v_tiles, att_tiles = {}, {}, {}, {}

            dbgkeep = []
            if KDBG in ("prep", "rs"):
                dbgkeep = [misc.tile([128, W], F32, name=f"dk{i}")
                           for i in range(NW)]

            # phase 1 + phase 2 share one PSUM budget (8 banks):
            # qkv/ph3 ring 2 x 1 + scores 2 x 2 + psas 2 x 1 = 8
            with (
                tc.tile_pool(name=f"pqk{rep}", bufs=2, space="PSUM") as pqk,
                tc.tile_pool(name=f"pp{rep}", bufs=2, space="PSUM") as pp,
                tc.tile_pool(name=f"pap{rep}", bufs=2, space="PSUM") as pap,
            ):
                def warmup():
                    # hold the PE p-state up while the first DMAs land
                    nc.gpsimd.memset(warm_t[:], 0.0)
                    for i in range(40):
                        pw = pqk.tile([128, 128], F32, name=f"warm{i}",
                                      tag="pqk")
                        nc.tensor.matmul(pw[:], warm_t[:], warm_t[:],
                                         start=True, stop=True)
                        yield

                def phase1_window(w):
                    xtw = xtp.tile([128, NCI, W], BF16, name="xtw", tag="xtw")
                    if w == 0:
                        # HWDGE costs ~0.6us flat per DMA: few, mid-size
                        # chunks ordered so the first chains start early
                        nc.sync.dma_start(wqt[:, 0:4, :], wq_in[:, 0:4, :])
                        nc.sync.dma_start(xtw[:, 0:2, :], xt_in[:, w, 0:2, :])
                        nc.sync.dma_start(wqt[:, 4:, :], wq_in[:, 4:, :])
                        nc.sync.dma_start(xtw[:, 2:4, :], xt_in[:, w, 2:4, :])
                        nc.sync.dma_start(wkt[:], wk_in[:])
                        nc.sync.dma_start(xtw[:, 4:6, :], xt_in[:, w, 4:6, :])
                        nc.sync.dma_start(wvt[:], wv_in[:])
                        nc.sync.dma_start(xtw[:, 6:, :], xt_in[:, w, 6:, :])
                        nc.sync.dma_start(vones_t[:], vones_in[:])
                    elif w == 1:
                        nc.sync.dma_start(xtw[:, 0:4, :], xt_in[:, w, 0:4, :])
                        nc.sync.dma_start(xtw[:, 4:, :], xt_in[:, w, 4:, :])
                        nc.sync.dma_start(tri_t[:], tri_in[:])
                    else:
                        nc.sync.dma_start(xtw[:, 0:4, :], xt_in[:, w, 0:4, :])
                        nc.sync.dma_start(xtw[:, 4:, :], xt_in[:, w, 4:, :])
                    if w == 5:
                        nc.sync.dma_start(w3t[:], w3_in[:])

                    for fam, wt, dst in (("q", wqt, qt_tiles),
                                         ("k", wkt, kt_tiles)):
                        ps = pqk.tile([128, W], F32, name=f"ps_{fam}{w}",
                                      tag="pqk")
                        for ci in range(NCI):
                            nc.tensor.matmul(ps[:], wt[:, ci, :],
                                             xtw[:, ci, :],
                                             start=(ci == 0),
                                             stop=(ci == NCI - 1))
                            if ci % 4 == 3:
                                yield
                        sb = (qtp if fam == "q" else ktp).tile(
                            [128, W], BF16, name=f"{fam}t{w}", tag=f"{fam}t")
                        nc.vector.tensor_copy(sb[:], ps[:])
                        dst[w] = sb

                    # V directly in [k-token, head*dim] orientation:
                    # lhsT = x^T 128-token slice, rhs = wv chunk.
                    vw = vp.tile([128, 4, 130], BF16, name=f"vw{w}", tag="vw")
                    vwr = vw.rearrange("p k (l e) -> p k l e", e=65)
                    for kcl in range(4):
                        psv = pqk.tile([128, W], F32, name=f"psv{w}_{kcl}",
                                       tag="pqk")
                        for ci in range(NCI):
                            nc.tensor.matmul(
                                psv[:, 0:128],
                                xtw[:, ci, KC * kcl:KC * (kcl + 1)],
                                wvt[:, ci, :],
                                start=(ci == 0), stop=(ci == NCI - 1))
                        # both heads in one strided copy (PSUM: DVE only)
                        nc.vector.tensor_copy(vwr[:, kcl, :, 0:64],
                                              psv[:, 0:128].rearrange(
                                                  "p (l e) -> p l e", e=64))
                        yield
                    nc.gpsimd.tensor_copy(vwr[:, :, :, 64], vones_t[:])
                    v_tiles[w] = vw

                    aw = attp.tile([128, W], BF16, name=f"attw{w}", tag="attw")
                    att_tiles[w] = aw

                def emit_tail(tb, tj, tqw, tpsas):
                    preps = []
                    for l in range(HPC):
                        rc = rcpool.tile([1, W], F32R,
                                         name=f"rc{tb}{tj}{l}", tag="rc")
                        with nc.allow_low_precision(reason="f32r recip"):
                            nc.vector.reciprocal(rc[:], tpsas[l][64:65, :])
                        prep = rcpool.tile([64, W], F32R,
                                           name=f"prs{tb}{tj}{l}",
                                           tag=f"prs{l}")
                        nc.gpsimd.partition_broadcast(prep[:], rc[:])
                        preps.append(prep)
                    if KDBG == "prep":
                        for l in range(HPC):
                            nc.vector.tensor_copy(
                                dbgkeep[tqw][64 * l:64 * l + 64, :],
                                preps[l][:])
                    if KDBG == "rs":
                        for l in range(HPC):
                            nc.vector.tensor_copy(
                                dbgkeep[tqw][32 * l:32 * l + 1, :],
                                tpsas[l][64:65, :])
                    for l in range(HPC):
                        nc.vector.scalar_tensor_tensor(
                            att_tiles[tqw][64 * l:64 * l + 64, :],
                            tpsas[l][0:64, :], 1.0,
                            preps[l][:],
                            MULT, MULT)
                    # both half-window slots in one DMA: dst [p][s,cc]
                    # via dim permutation matches src [p][h,cc] order
                    nc.sync.dma_start(
                        cc_in[tb][2 * tj:2 * tj + 2, :, :]
                        .rearrange("s p c -> p s c"),
                        att_tiles[tqw][:].rearrange(
                            "p (h c) -> p h c", c=HQS))

                def collective(g):
                    if not knocc:
                        nc.gpsimd.collective_compute(
                            "AllToAll", mybir.AluOpType.bypass,
                            ins=[cc_in[g].opt()], outs=[cc_out[g].opt()],
                            replica_groups=[list(range(NCORES))])

                state = {"pending": None}
                att7_anchors = []

                def attention_window(qw, post_tail=None):
                    """Yields one unit per (pair, head). Chunk pair order:
                    off-diagonal ascending, then diagonal pairs in
                    DESCENDING kcl so the first (full-width) matmul
                    initializes the psas accumulator region."""
                    b, j = qw // WPB, qw % WPB
                    nk = 4 * (j + 1)
                    # pair list: (kca, kcb, widths) in processing order
                    pairs = [(2 * k2, 2 * k2 + 1, W, W)
                             for k2 in range(2 * j)]
                    # diagonal: kcl 3,2 then 1,0 (prefix widths 128*(kcl+1))
                    pairs.append((4 * j + 3, 4 * j + 2, W, 384))
                    pairs.append((4 * j + 1, 4 * j + 0, 256, 128))
                    npairs = len(pairs)
                    psas = [pap.tile([65, W], F32, name=f"psa{b}{l}{j}",
                                     tag="psa") for l in range(HPC)]
                    for pi, (kca, kcb, wa, wb) in enumerate(pairs):
                        diag = kca >= 4 * j
                        for l in range(HPC):
                            pss = pp.tile([128, 2, W], F32,
                                          name=f"pss{b}{l}{j}{pi}",
                                          tag="pp2")
                            for half, kc, wd in ((0, kca, wa), (1, kcb, wb)):
                                kw = WPB * b + kc // 4
                                kcl = kc % 4
                                nc.tensor.matmul(
                                    pss[:, half, 0:wd],
                                    kt_tiles[kw][64 * l:64 * l + 64,
                                                 KC * kcl:KC * (kcl + 1)],
                                    qt_tiles[qw][64 * l:64 * l + 64, 0:wd],
                                    start=True, stop=True)
                            pb = pbp.tile([128, 2, W], BF16,
                                          name=f"pb{b}{l}{j}{pi}",
                                          tag="pb")
                            if not diag:
                                nc.scalar.activation(
                                    pb.rearrange("p h f -> p (h f)"),
                                    pss.rearrange("p h f -> p (h f)"),
                                    EXP)
                            else:
                                for half, kc, wd in ((0, kca, wa),
                                                     (1, kcb, wb)):
                                    nc.scalar.activation(
                                        pb[:, half, 0:wd],
                                        pss[:, half, 0:wd], EXP)
                                    # upper-tri mask on the 128-col
                                    # boundary block of this chunk
                                    kcl = kc % 4
                                    blk = pb[:, half,
                                             KC * kcl:KC * (kcl + 1)]
                                    nc.vector.tensor_mul(blk, blk, tri_t[:])
                            first = pi == 0
                            last = pi == npairs - 1
                            for half, kc, wd in ((0, kca, wa), (1, kcb, wb)):
                                kw = WPB * b + kc // 4
                                kcl = kc % 4
                                mm = nc.tensor.matmul(
                                    psas[l][:, 0:wd],
                                    v_tiles[kw][:, kcl,
                                                65 * l:65 * l + 65],
                                    pb[:, half, 0:wd],
                                    start=(first and half == 0),
                                    stop=(last and half == 1))
                            if qw == NW - 1:
                                att7_anchors.append(mm)
                            yield
                        if pi == 0 and state["pending"] is not None:
                            emit_tail(*state["pending"])
                            state["pending"] = None
                            if post_tail is not None:
                                post_tail()
                    state["pending"] = (b, j, qw, psas)

                def anchor(inst, idx, why):
                    if att7_anchors:
                        a = att7_anchors[min(idx, len(att7_anchors) - 1)]
                        bass._add_dep_helper(inst.ins, a.ins, sync=False,
                                             reason=why)

                def phase3_loads(rt_tiles, g, anchor0=None):
                    rt_src = cc_in[g] if knocc else cc_out[g]
                    # one DMA for all 8 source slots: dst [p][s, cc],
                    # src viewed [p][s][cc] via dim permutation -- saves
                    # 7 HWDGE issue slots (~0.65us each) at the phase-3
                    # boundary
                    rta = finp.tile([128, NCORES, GCW[g]], BF16,
                                    name=f"rta{g}", tag="rta", bufs=2)
                    for h in range(4):
                        ld = nc.sync.dma_start(
                            rta[:, 2 * h:2 * h + 2, :],
                            rt_src[2 * h:2 * h + 2].rearrange(
                                "s p c -> p s c"))
                        if anchor0 is not None:
                            anchor(ld, anchor0 + h, "late rt load placement")
                    for s in range(NCORES):
                        rt_tiles.append(rta[:, s, :])
                    yield

                def phase3_units(rt_tiles, out_r, g, use_act, anchor0=None):
                    # ec chunks in pairs sharing one ot tile and ONE
                    # output DMA (dst [p][e,c] via dim permutation) --
                    # halves the tail's HWDGE issue chain
                    co, cw = GCOL[g], GCW[g]
                    for e2 in range(NCI // 2):
                        ot = otp.tile([128, 2, cw], F32,
                                      name=f"ot{g}_{e2}", tag="ot")
                        for half in range(2):
                            ec = 2 * e2 + half
                            pso = pqk.tile([128, cw], F32,
                                           name=f"pso{g}_{ec}", tag="pqk")
                            for s in range(NCORES):
                                mm = nc.tensor.matmul(
                                    pso[:],
                                    w3t[:, s, KC * ec:KC * (ec + 1)],
                                    rt_tiles[s],
                                    start=(s == 0), stop=(s == NCORES - 1))
                                if anchor0 is not None and s == 0:
                                    anchor(mm, anchor0 + ec,
                                           "ph3a interleave")
                            if use_act and half == 1:
                                nc.scalar.copy(ot[:, half, :], pso[:])
                            else:
                                nc.vector.tensor_copy(ot[:, half, :], pso[:])
                        nc.sync.dma_start(
                            out_r[2 * e2:2 * e2 + 2][:, :, co:co + cw]
                            .rearrange("e p c -> p e c"),
                            ot[:])
                        yield

                def drain(gen):
                    for _ in gen:
                        pass

                def chain(*gens):
                    for g in gens:
                        yield from g

                def imerge(primary, fill, n_primary, n_fill):
                    """Advance `fill` n_fill units spread across primary's
                    units; drains primary, leaves extra fill for later."""
                    done_f = 0
                    i = 0
                    for _ in primary:
                        i += 1
                        want = min(n_fill, (i * n_fill + n_primary - 1)
                                   // n_primary)
                        while done_f < want:
                            if next(fill, StopIteration) is StopIteration:
                                done_f = n_fill
                                break
                            done_f += 1
                    while done_f < n_fill:
                        if next(fill, StopIteration) is StopIteration:
                            break
                        done_f += 1

                def att_units(qw):
                    j = qw % WPB
                    return (2 * j + 2) * HPC

                # ---------- schedule ----------
                out_r = out_t.ap().rearrange("(e p) f -> e p f", p=128)
                rt_a, rt_b = [], []
                drain(warmup())
                drain(phase1_window(0))
                drain(phase1_window(1))
                # fill chain: remaining phase-1 windows. NOTHING that
                # waits on a collective may be interleaved into the
                # attention stream: engine SEQs evaluate waits in-order,
                # so one waiting DMA (or matmul) stalls everything
                # emitted after it on that engine.
                fills = chain(
                    phase1_window(2), phase1_window(3), phase1_window(4),
                    phase1_window(5), phase1_window(6), phase1_window(7),
                )
                budgets = [8, 8, 8, 8, 2, 6, 8, 8]
                posts = {WPB: lambda: collective(0)}
                for qw in range(NW):
                    imerge(attention_window(qw, post_tail=posts.get(qw)),
                           fills, att_units(qw), budgets[qw])
                drain(fills)
                if state["pending"] is not None:
                    emit_tail(*state["pending"])
                    state["pending"] = None
                collective(1)

                # ---------- debug dumps ----------
                if KDBG:
                    out_r = out_t.ap().rearrange("(e p) f -> e p f", p=128)
                    if KDBG in ("prep", "rs"):
                        for w in range(NW):
                            nc.sync.dma_start(out_r[w], dbgkeep[w][:])
                    else:
                        dbg_src = {"q": qt_tiles, "k": kt_tiles,
                                   "att": att_tiles, "v": v_tiles}[KDBG]
                        with tc.tile_pool(name=f"dbgp{rep}", bufs=2) as dbgp:
                            for w in range(NW):
                                st = dbgp.tile([128, W], F32, name=f"dbg{w}",
                                               tag="dbg")
                                if KDBG == "v":
                                    nc.vector.tensor_copy(
                                        st[:], dbg_src[w][:].rearrange(
                                            "p a b -> p (a b)")[:, 0:W])
                                else:
                                    nc.vector.tensor_copy(st[:], dbg_src[w][:])
                                nc.sync.dma_start(out_r[w], st[:])
                        continue

                if not KDBG:
                    # phase3a: anchored into attention 7's exp-bound tail
                    # (collective 0 landed long before). Anchors both stop
                    # the scheduler from hoisting the loads so early that
                    # their collective wait blocks the SP DMA queue, and
                    # let the pso matmuls fill attention-7 exp bubbles.
                    drain(phase3_loads(rt_a, 0, anchor0=0))
                    drain(phase3_units(rt_a, out_r, 0, use_act=True,
                                       anchor0=8))
                    # phase3b waits on the last collective: anchored after
                    # attention 7's last unit (relative, so KREP-unrolled
                    # reps each keep their own tail in place)
                    drain(phase3_loads(rt_b, 1, anchor0=99))
                    drain(phase3_units(rt_b, out_r, 1, use_act=True,
                                       anchor0=99))

    nc.compile()
    _NC_CACHE[key] = nc
    return nc


def prep_inputs(x, wq, wk, wv, wp1, wp2):
    """Host-side sharding / layout prep. Returns per-core input dicts."""
    bf16 = mybir.dt.np(BF16)
    x = np.asarray(x, np.float32)
    wq = np.asarray(wq, np.float32)
    wk = np.asarray(wk, np.float32)
    wv = np.asarray(wv, np.float32)
    wp1 = np.asarray(wp1, np.float32)
    wp2 = np.asarray(wp2, np.float32)

    # x^T in [p, w, ci, f] layout; token order REVERSED inside each window
    xtf = x.reshape(TT, C).T                      # [C, TT]
    xt4 = xtf.reshape(NCI, 128, NW, W)[:, :, :, ::-1]
    xt_host = np.ascontiguousarray(
        xt4.transpose(1, 2, 0, 3)).astype(bf16)

    scale = 1.0 / np.sqrt(D)

    def wlay(wm):                                  # [C, 128] -> [128, NCI, 128]
        return np.ascontiguousarray(
            wm.reshape(NCI, 128, 128).transpose(1, 0, 2)).astype(bf16)

    w3 = wp1.reshape(C, C) @ wp2                   # [hd, e]
    w3_host = np.ascontiguousarray(
        w3.reshape(NCORES, 128, C).transpose(1, 0, 2)).astype(bf16)

    # keep where q'(col) <= k'(partition)
    p = np.arange(128)[:, None]
    f = np.arange(KC)[None, :]
    tri_host = np.ascontiguousarray((f <= p).astype(np.float32)).astype(bf16)
    vones_host = np.ones((128, 4, 2), np.float32).astype(bf16)

    in_maps = []
    for c in range(NCORES):
        h0 = HPC * c
        wq_c = wlay(wq[:, h0:h0 + HPC, :].reshape(C, HPC * D) * scale)
        wk_c = wlay(wk[:, h0:h0 + HPC, :].reshape(C, HPC * D))
        wv_c = wlay(wv[:, h0:h0 + HPC, :].reshape(C, HPC * D))
        in_maps.append({
            "xt": xt_host, "wq": wq_c, "wk": wk_c, "wv": wv_c,
            "w3": w3_host, "tri": tri_host,
            "vones": vones_host,
        })
    return in_maps


def assemble_output(results):
    # Collective group g gives core r a cw-column slice of window
    # j_base + r // (W//cw), slot q = r % (W//cw), att cols
    # [cw*q, cw*(q+1)) -- columns in reversed token order.
    parts = [(0, 0, HQS, 0),            # (batch, j_base, cw, out_t col0)
             (1, 0, HQS, HQS)]
    out = np.empty((TT, C), np.float32)
    for r in range(NCORES):
        ot = results[r]["out_t"]
        for b, j_base, cw, co in parts:
            spw = W // cw
            j = j_base + r // spw
            q = r % spw
            g0 = T * b + W * j + W - cw * (q + 1)
            out[g0:g0 + cw, :] = ot[:, co:co + cw][:, ::-1].T
    return out.reshape(B, T, C)


def kernel(x, wq, wk, wv, wp1, wp2):
    in_maps = prep_inputs(x, wq, wk, wv, wp1, wp2)
    nc = build_nc()
    res = run_bass_kernel_spmd(nc, in_maps, list(range(NCORES)))
    return assemble_output(res.results)
